# revision 8
# baseline (speedup 1.0000x reference)
"""DN4 retrieval-kNN kernel for Trainium2 (8 NeuronCores, SPMD, no collectives).

Relu-fold with PSUM accumulation, pipelined in 552-pair half-units. Host
prepares the replicated class-descriptor bank (grouped, L2-normalized,
transposed to [C, n]); each way's 2208 padded columns pair col j with col
j+1104 and are stored as [delta_h | b_h] per half-unit, delta_j = d_j -
d_{j+1104}, b_j = d_{j+1104}. On device, per half-unit (552 pairs, one
2-bank PSUM tile, 4 in flight):

  PE:  delta-sims -> pR psum fp32 (q . delta)
  ACT: relu(pR) -> pR IN PLACE
  PE:  b-sims ACCUMULATE onto pR (start=False)  -> pR = b + relu(a-b)
                                                 = max(a, b) exactly
  DVE: max8 over the 552 pair-maxes; the two halves' top-8s merge with a
       16-wide max8. Top-3 of the 1104 pair-maxes == top-3 of the way's
       2205 sims unless >=2 of the top-3 share a pair (P ~ 3/1104 per
       row; measured 2.1e-4 rel err vs the 2e-2 tolerance).

DVE (max8 at 1 elem/lane/cycle) is the pacing engine; the emission is
software-pipelined one half-unit ahead so the in-order PE queue never
waits on a relu. Queries are host-pre-transposed; 1/|q| and 1/(441*3)
live in the host-built amask, applied by the per-way score matmuls into
a borrowed psum tile; host sums m-tiles and cores. All inputs ship as
one packed fp16 blob in 4 staged DMAs (each dma_start costs ~2.5us
fixed); a few junk matmuls warm the PE through its p-state ramp.
"""
import os
import sys

import numpy as np

for _p in ('/opt/trn_rl_repo', '/root/.axon_site/_ro/trn_rl_repo'):
    if os.path.isdir(_p) and _p not in sys.path:
        sys.path.insert(0, _p)

WAYS, SHOTS, Q = 5, 5, 30
C, HW = 128, 441
K = 3
NWAY = SHOTS * HW            # 2205 support descriptors per way
WPAD = 2208                  # per-way padded width (3 zero descriptors)
HALF = WPAD // 2             # 1104 pairs per way
ND = WAYS * WPAD             # 11040
DT = 87                      # bank column-tiles of 128
ND_PAD = DT * 128            # 11136
NCORES = 8
TROWS = Q * HW               # 13230 query-descriptor rows in total
RPC = (TROWS + NCORES - 1) // NCORES   # 1654 rows per core
MT = (RPC + 127) // 128      # 13 m-tiles per core
M_PAD = MT * 128             # 1664
SLOTS = 8                    # local query slots a core's rows can span (<=5)

QUART = HALF // 2            # 552

# one packed input tensor, staged dma_starts (each ~2.5us fixed):
# [zqt_t0 | bank_way0 | zqt_rest | bank_rest | amask16]
OFF_ZQT0 = 0
OFF_BANK0 = OFF_ZQT0 + 128
OFF_ZQTR = OFF_BANK0 + WPAD
OFF_BANKR = OFF_ZQTR + (MT - 1) * 128
OFF_AM = OFF_BANKR + (ND_PAD - WPAD)
BLOB = OFF_AM + 2 * MT * SLOTS

_CACHE = {}


def _build_program():
    import concourse.bacc as bacc
    import concourse.mybir as mybir
    from concourse import tile

    dt = mybir.dt
    AF = mybir.ActivationFunctionType
    ALU = mybir.AluOpType
    AX = mybir.AxisListType

    nc = bacc.Bacc('TRN2', target_bir_lowering=False, debug=False)

    d_blob = nc.dram_tensor('blob', [128, BLOB], dt.float16, kind='ExternalInput')
    d_out = nc.dram_tensor('scores', [SLOTS, WAYS * MT], dt.float32,
                           kind='ExternalOutput')

    with tile.TileContext(nc) as tc:
        with tc.tile_pool(name='persist', bufs=1) as pp, \
             tc.tile_pool(name='work', bufs=3) as wp:

            blob = pp.tile([128, BLOB], dt.float16)

            def zqt(t):
                if t == 0:
                    return blob[:, OFF_ZQT0:OFF_ZQT0 + 128]
                o = OFF_ZQTR + (t - 1) * 128
                return blob[:, o:o + 128]

            def bankw(w, lo, hi):
                if w == 0:
                    return blob[:, OFF_BANK0 + lo:OFF_BANK0 + hi]
                o = OFF_BANKR + (w - 1) * WPAD
                return blob[:, o + lo:o + hi]

            amask3 = blob[:, OFF_AM:OFF_AM + 2 * MT * SLOTS].bitcast(
                dt.float32).rearrange('p (t s) -> p t s', t=MT)
            scw = pp.tile([SLOTS, WAYS, MT], dt.float32)

            # ---- input DMAs, staged so unit (0,0) starts asap ----
            nc.sync.dma_start(blob[:, 0:OFF_BANK0 + HALF],
                              d_blob[:, 0:OFF_BANK0 + HALF])
            nc.sync.dma_start(blob[:, OFF_BANK0 + HALF:OFF_ZQTR],
                              d_blob[:, OFF_BANK0 + HALF:OFF_ZQTR])
            nc.sync.dma_start(blob[:, OFF_ZQTR:OFF_BANKR],
                              d_blob[:, OFF_ZQTR:OFF_BANKR])
            nc.sync.dma_start(blob[:, OFF_BANKR:BLOB],
                              d_blob[:, OFF_BANKR:BLOB])

            with tc.tile_pool(name='ps', bufs=4, space='PSUM') as ps:

                halves = [(w, t, h) for w in range(WAYS) for t in range(MT)
                          for h in range(2)]
                # warm the PE through its p-state ramp during the input DMA
                junk16 = pp.tile([128, C], dt.float16, name='junk16w')
                nc.gpsimd.memset(junk16[:], 0.0)
                warm = ps.tile([128, QUART], dt.float32, tag='pR',
                               name='warm')
                for _ in range(12):
                    nc.tensor.matmul(warm[:, 0:128], junk16[:], junk16[:],
                                     start=True, stop=True)
                m8bigs, m16s, pRs = {}, {}, {}
                pending = []

                def emit_front(i):
                    w, t, h = halves[i]
                    pR = ps.tile([128, QUART], dt.float32, tag='pR',
                                 name=f'pR_{i}')
                    pRs[i] = pR
                    base = h * HALF
                    for off, sz in ((0, 512), (512, 40)):
                        nc.tensor.matmul(pR[:, off:off + sz], zqt(t),
                                         bankw(w, base + off, base + off + sz),
                                         start=True, stop=True)
                    nc.scalar.activation(pR[:], pR[:], AF.Relu)

                def emit_back(i):
                    w, t, h = halves[i]
                    pR = pRs.pop(i)
                    if t == 0 and h == 0:
                        m8bigs[w] = wp.tile([128, MT, 8], dt.float32, tag='m8',
                                            name=f'm8_{w}')
                    if h == 0:
                        m16s[w, t] = wp.tile([128, 2, 8], dt.float32,
                                             tag='m16', name=f'm16_{i}')
                    base = h * HALF + QUART
                    # b-sims accumulate onto relu(delta): pR = max(a, b)
                    for off, sz in ((0, 512), (512, 40)):
                        nc.tensor.matmul(pR[:, off:off + sz], zqt(t),
                                         bankw(w, base + off, base + off + sz),
                                         start=False, stop=True)
                    nc.vector.max(m16s[w, t][:, h, :], pR[:])
                    if h == 1:
                        m16 = m16s.pop((w, t))
                        nc.vector.max(m8bigs[w][:, t, :],
                                      m16[:].rearrange('p a b -> p (a b)'))
                        if t == MT - 1:
                            pending.append(w)

                def emit_wayend():
                    w = pending.pop(0)
                    m8big = m8bigs.pop(w)
                    stv = wp.tile([128, MT], dt.float32, tag='stv')
                    nc.vector.reduce_sum(stv[:], m8big[:, :, 0:K], axis=AX.X)
                    # borrow a rotating psum tile for this way's 13 tiny
                    # score matmuls, then stash the [SLOTS, MT] result in SBUF
                    sc = ps.tile([128, QUART], dt.float32, tag='pR',
                                 name=f'sc_{w}')
                    for tt in range(MT):
                        nc.tensor.matmul(sc[0:SLOTS, tt:tt + 1],
                                         amask3[:, tt, :], stv[:, tt:tt + 1],
                                         start=True, stop=True)
                    nc.scalar.activation(scw[:, w, :], sc[0:SLOTS, 0:MT],
                                         AF.Copy)

                # software-pipelined so the in-order PE queue never waits on
                # a relu; 4 psum bufs keep ~3 half-units in flight
                for i in range(len(halves) + 1):
                    if i < len(halves):
                        emit_front(i)
                    if i >= 1:
                        emit_back(i - 1)
                    if pending and (i - 1 >= len(halves) - 1 or
                                    (i >= 8 and halves[i - 8][1] == MT - 1
                                     and halves[i - 8][2] == 1)):
                        emit_wayend()
                while pending:
                    emit_wayend()
            # host sums the MT axis (and across cores)
            nc.sync.dma_start(d_out[:], scw[:].rearrange('s w t -> s (w t)'))

    nc.finalize()
    return nc


def _host_prep(support_images, support_labels, query_images):
    support_images = np.ascontiguousarray(np.asarray(support_images, np.float32))
    support_labels = np.asarray(support_labels, np.float32)
    query_images = np.ascontiguousarray(np.asarray(query_images, np.float32))

    labels = np.argmax(support_labels, axis=1)
    order = np.argsort(labels, kind='stable')
    sup = support_images[order].reshape(WAYS * SHOTS, C, HW)

    # replicated class-descriptor bank: grouped, fp16, L2-normalized over C
    # (norms from the fp16-rounded values the matmuls see), padded per way
    desc = sup.transpose(0, 2, 1).reshape(WAYS, NWAY, C).astype(np.float16)
    dn = np.sqrt((desc.astype(np.float32) ** 2).sum(-1, keepdims=True) + 1e-4)
    dhat = (desc.astype(np.float32) / dn)
    dpad = np.zeros((WAYS, WPAD, C), np.float32)
    dpad[:, :NWAY] = dhat
    # [delta_h0 | b_h0 | delta_h1 | b_h1] per way: half-unit h covers pairs
    # (j, j+HALF) for j in [h*QUART, (h+1)*QUART)
    delta = dpad[:, :HALF] - dpad[:, HALF:]
    bvals = dpad[:, HALF:]
    bankw = np.concatenate(
        [delta[:, :QUART], bvals[:, :QUART],
         delta[:, QUART:], bvals[:, QUART:]], axis=1)
    flat = bankw.reshape(ND, C)
    flat = np.concatenate([flat, np.zeros((ND_PAD - ND, C), np.float32)], 0)
    bank_dev = flat.T.astype(np.float16)                         # [C, ND_PAD]

    # flat query-descriptor rows [13230, C], row r = (q = r//441, hw = r%441)
    zq_flat = query_images.reshape(Q, C, HW).transpose(0, 2, 1).reshape(TROWS, C)
    blob_devs = []
    for core in range(NCORES):
        r0 = core * RPC
        zb = zq_flat[r0:r0 + RPC]
        zb = np.concatenate(
            [zb, np.zeros((M_PAD - zb.shape[0], C), np.float32)], 0)
        zqt_dev = zb.T.reshape(C, MT * 128).astype(np.float16)
        # 1/|q| per padded row (from the fp16 values the matmuls see),
        # folded into the amask weights
        q16 = zb.astype(np.float16).astype(np.float32)
        qn = np.sqrt((q16 ** 2).sum(1) + 1e-4)
        q0 = r0 // HW
        amask = np.zeros((128, MT, SLOTS), np.float32)
        lr = np.arange(MT * 128)
        r = r0 + lr
        valid = (lr < RPC) & (r < TROWS)
        amask[lr[valid] % 128, lr[valid] // 128, (r[valid] // HW) - q0] = \
            1.0 / (HW * K * qn[lr[valid]])
        am16 = amask.reshape(128, MT * SLOTS).view(np.float16)
        blob = np.concatenate(
            [zqt_dev[:, 0:128], bank_dev[:, 0:WPAD], zqt_dev[:, 128:],
             bank_dev[:, WPAD:], am16], axis=1)
        blob_devs.append(np.ascontiguousarray(blob))
    return blob_devs


def kernel(support_images, support_labels, query_images):
    from concourse import bass_utils

    if 'nc' not in _CACHE:
        _CACHE['nc'] = _build_program()
    nc = _CACHE['nc']

    blob_devs = _host_prep(support_images, support_labels, query_images)

    in_maps = [{'blob': blob_devs[c]} for c in range(NCORES)]
    try:
        res = bass_utils.run_bass_kernel_spmd(
            nc, in_maps, core_ids=list(range(NCORES)))
    except Exception:
        # transient NRT/tunnel failures happen; one retry
        import time
        time.sleep(2.0)
        res = bass_utils.run_bass_kernel_spmd(
            nc, in_maps, core_ids=list(range(NCORES)))
    scores = np.zeros((Q, WAYS), np.float32)
    for c in range(NCORES):
        q0 = (c * RPC) // HW
        part = res.results[c]['scores'].reshape(SLOTS, WAYS, MT).sum(2)
        for s in range(SLOTS):
            if q0 + s < Q:
                scores[q0 + s] += part[s]
    return scores.astype(np.float32)


# revision 9
# speedup vs baseline: 1.0306x; 1.0306x over previous
"""DN4 retrieval-kNN kernel for Trainium2 (8 NeuronCores, SPMD, no collectives).

v5: relu-fold with PSUM accumulation, half-unit pipelining. Host prepares the replicated
class-descriptor bank (grouped, L2-normalized, transposed to [C, n]) with
each way's 2208 padded columns stored as [delta | b]: delta_j =
d_j - d_{j+1104}, b_j = d_{j+1104}. On device, per (way, m-tile) unit:

  PE:  delta-sims -> pR psum fp32                 (q . delta, 1104 cols)
  ACT: relu(pR) -> pR IN PLACE                    (one pass, half the old)
  PE:  b-sims ACCUMULATE onto pR (start=False)    -> pR = b + relu(a-b)
                                                   = max(a, b) exactly
  DVE: max8 over the 1104 pair-maxes in pR, top-3 of those == top-3 of
       the way's 2205 sims unless >=2 of the top-3 share a pair
       (P ~ 3/1104 per row; error way under the 2e-2 tolerance)

DVE (max8 at 1 elem/lane/cycle) is the pacing engine: ~1.37us x 65 units.
Queries are host-pre-transposed; 1/|q| and 1/(441*3) live in the host-built
amask, applied by the per-way score matmuls; host sums m-tiles and cores.
"""
import os
import sys

import numpy as np

for _p in ('/opt/trn_rl_repo', '/root/.axon_site/_ro/trn_rl_repo'):
    if os.path.isdir(_p) and _p not in sys.path:
        sys.path.insert(0, _p)

WAYS, SHOTS, Q = 5, 5, 30
C, HW = 128, 441
K = 3
NWAY = SHOTS * HW            # 2205 support descriptors per way
WPAD = 2208                  # per-way padded width (3 zero descriptors)
HALF = WPAD // 2             # 1104 pairs per way
ND = WAYS * WPAD             # 11040
DT = 87                      # bank column-tiles of 128
ND_PAD = DT * 128            # 11136
NCORES = 8
TROWS = Q * HW               # 13230 query-descriptor rows in total
RPC = (TROWS + NCORES - 1) // NCORES   # 1654 rows per core
MT = (RPC + 127) // 128      # 13 m-tiles per core
M_PAD = MT * 128             # 1664
SLOTS = 8                    # local query slots a core's rows can span (<=5)

QUART = HALF // 2            # 552

# one packed input tensor, staged dma_starts (each ~2.5us fixed):
# [zqt_t0 | bank_way0 | zqt_rest | bank_rest | amask16]
OFF_ZQT0 = 0
OFF_BANK0 = OFF_ZQT0 + 128
OFF_ZQTR = OFF_BANK0 + WPAD
OFF_BANKR = OFF_ZQTR + (MT - 1) * 128
OFF_AM = OFF_BANKR + (ND_PAD - WPAD)
BLOB = OFF_AM + 2 * MT * SLOTS

_CACHE = {}


def _build_program():
    import concourse.bacc as bacc
    import concourse.mybir as mybir
    from concourse import tile

    dt = mybir.dt
    AF = mybir.ActivationFunctionType
    ALU = mybir.AluOpType
    AX = mybir.AxisListType

    nc = bacc.Bacc('TRN2', target_bir_lowering=False, debug=False)

    d_blob = nc.dram_tensor('blob', [128, BLOB], dt.float16, kind='ExternalInput')
    d_out = nc.dram_tensor('scores', [SLOTS, WAYS * MT], dt.float32,
                           kind='ExternalOutput')

    with tile.TileContext(nc) as tc:
        with tc.tile_pool(name='persist', bufs=1) as pp, \
             tc.tile_pool(name='work', bufs=3) as wp:

            blob = pp.tile([128, BLOB], dt.float16)

            def zqt(t):
                if t == 0:
                    return blob[:, OFF_ZQT0:OFF_ZQT0 + 128]
                o = OFF_ZQTR + (t - 1) * 128
                return blob[:, o:o + 128]

            def bankw(w, lo, hi):
                if w == 0:
                    return blob[:, OFF_BANK0 + lo:OFF_BANK0 + hi]
                o = OFF_BANKR + (w - 1) * WPAD
                return blob[:, o + lo:o + hi]

            amask3 = blob[:, OFF_AM:OFF_AM + 2 * MT * SLOTS].bitcast(
                dt.float32).rearrange('p (t s) -> p t s', t=MT)
            scw = pp.tile([SLOTS, WAYS, MT], dt.float32)

            # ---- input DMAs, staged so unit (0,0) starts asap ----
            nc.sync.dma_start(blob[:, 0:OFF_BANK0 + HALF],
                              d_blob[:, 0:OFF_BANK0 + HALF])
            nc.sync.dma_start(blob[:, OFF_BANK0 + HALF:OFF_ZQTR],
                              d_blob[:, OFF_BANK0 + HALF:OFF_ZQTR])
            nc.sync.dma_start(blob[:, OFF_ZQTR:OFF_BANKR],
                              d_blob[:, OFF_ZQTR:OFF_BANKR])
            nc.sync.dma_start(blob[:, OFF_BANKR:BLOB],
                              d_blob[:, OFF_BANKR:BLOB])

            with tc.tile_pool(name='ps', bufs=4, space='PSUM') as ps:

                halves = [(w, t, h) for w in range(WAYS) for t in range(MT)
                          for h in range(2)]
                # warm the PE through its p-state ramp during the input DMA
                junk16 = pp.tile([128, C], dt.float16, name='junk16w')
                nc.gpsimd.memset(junk16[:], 0.0)
                warm = ps.tile([128, QUART], dt.float32, tag='pR',
                               name='warm')
                for _ in range(12):
                    nc.tensor.matmul(warm[:, 0:128], junk16[:], junk16[:],
                                     start=True, stop=True)
                m8bigs, m16s, pRs = {}, {}, {}
                pending = []

                def emit_front(i):
                    w, t, h = halves[i]
                    pR = ps.tile([128, QUART], dt.float32, tag='pR',
                                 name=f'pR_{i}')
                    pRs[i] = pR
                    base = h * HALF
                    for off, sz in ((0, 512), (512, 40)):
                        nc.tensor.matmul(pR[:, off:off + sz], zqt(t),
                                         bankw(w, base + off, base + off + sz),
                                         start=True, stop=True)
                    nc.scalar.activation(pR[:], pR[:], AF.Relu)

                def emit_back(i):
                    w, t, h = halves[i]
                    pR = pRs.pop(i)
                    if t == 0 and h == 0:
                        m8bigs[w] = wp.tile([128, MT, 8], dt.float32, tag='m8',
                                            name=f'm8_{w}')
                    if h == 0:
                        m16s[w, t] = wp.tile([128, 2, 8], dt.float32,
                                             tag='m16', name=f'm16_{i}')
                    base = h * HALF + QUART
                    # b-sims accumulate onto relu(delta): pR = max(a, b)
                    for off, sz in ((0, 512), (512, 40)):
                        nc.tensor.matmul(pR[:, off:off + sz], zqt(t),
                                         bankw(w, base + off, base + off + sz),
                                         start=False, stop=True)
                    nc.vector.max(m16s[w, t][:, h, :], pR[:])
                    if h == 1:
                        m16 = m16s.pop((w, t))
                        nc.vector.max(m8bigs[w][:, t, :],
                                      m16[:].rearrange('p a b -> p (a b)'))
                        if t == MT - 1:
                            pending.append(w)

                def emit_wayend():
                    w = pending.pop(0)
                    m8big = m8bigs.pop(w)
                    stv = wp.tile([128, MT], dt.float32, tag='stv')
                    nc.vector.reduce_sum(stv[:], m8big[:, :, 0:K], axis=AX.X)
                    # borrow a rotating psum tile for this way's 13 tiny
                    # score matmuls, then stash the [SLOTS, MT] result in SBUF
                    sc = ps.tile([128, QUART], dt.float32, tag='pR',
                                 name=f'sc_{w}')
                    for tt in range(MT):
                        nc.tensor.matmul(sc[0:SLOTS, tt:tt + 1],
                                         amask3[:, tt, :], stv[:, tt:tt + 1],
                                         start=True, stop=True)
                    nc.scalar.activation(scw[:, w, :], sc[0:SLOTS, 0:MT],
                                         AF.Copy)

                # software-pipelined 3 half-units deep (matches the 4 psum
                # bufs) so the in-order PE queue never waits on a relu
                for i in range(len(halves) + 3):
                    if i < len(halves):
                        emit_front(i)
                    if i >= 3:
                        emit_back(i - 3)
                    if pending and (i - 3 >= len(halves) - 1 or
                                    (i >= 10 and halves[i - 10][1] == MT - 1
                                     and halves[i - 10][2] == 1)):
                        emit_wayend()
                while pending:
                    emit_wayend()
            # host sums the MT axis (and across cores)
            nc.sync.dma_start(d_out[:], scw[:].rearrange('s w t -> s (w t)'))

    nc.finalize()
    return nc


def _host_prep(support_images, support_labels, query_images):
    support_images = np.ascontiguousarray(np.asarray(support_images, np.float32))
    support_labels = np.asarray(support_labels, np.float32)
    query_images = np.ascontiguousarray(np.asarray(query_images, np.float32))

    labels = np.argmax(support_labels, axis=1)
    order = np.argsort(labels, kind='stable')
    sup = support_images[order].reshape(WAYS * SHOTS, C, HW)

    # replicated class-descriptor bank: grouped, fp16, L2-normalized over C
    # (norms from the fp16-rounded values the matmuls see), padded per way
    desc = sup.transpose(0, 2, 1).reshape(WAYS, NWAY, C).astype(np.float16)
    dn = np.sqrt((desc.astype(np.float32) ** 2).sum(-1, keepdims=True) + 1e-4)
    dhat = (desc.astype(np.float32) / dn)
    dpad = np.zeros((WAYS, WPAD, C), np.float32)
    dpad[:, :NWAY] = dhat
    # [delta_h0 | b_h0 | delta_h1 | b_h1] per way: half-unit h covers pairs
    # (j, j+HALF) for j in [h*QUART, (h+1)*QUART)
    delta = dpad[:, :HALF] - dpad[:, HALF:]
    bvals = dpad[:, HALF:]
    bankw = np.concatenate(
        [delta[:, :QUART], bvals[:, :QUART],
         delta[:, QUART:], bvals[:, QUART:]], axis=1)
    flat = bankw.reshape(ND, C)
    flat = np.concatenate([flat, np.zeros((ND_PAD - ND, C), np.float32)], 0)
    bank_dev = flat.T.astype(np.float16)                         # [C, ND_PAD]

    # flat query-descriptor rows [13230, C], row r = (q = r//441, hw = r%441)
    zq_flat = query_images.reshape(Q, C, HW).transpose(0, 2, 1).reshape(TROWS, C)
    blob_devs = []
    for core in range(NCORES):
        r0 = core * RPC
        zb = zq_flat[r0:r0 + RPC]
        zb = np.concatenate(
            [zb, np.zeros((M_PAD - zb.shape[0], C), np.float32)], 0)
        zqt_dev = zb.T.reshape(C, MT * 128).astype(np.float16)
        # 1/|q| per padded row (from the fp16 values the matmuls see),
        # folded into the amask weights
        q16 = zb.astype(np.float16).astype(np.float32)
        qn = np.sqrt((q16 ** 2).sum(1) + 1e-4)
        q0 = r0 // HW
        amask = np.zeros((128, MT, SLOTS), np.float32)
        lr = np.arange(MT * 128)
        r = r0 + lr
        valid = (lr < RPC) & (r < TROWS)
        amask[lr[valid] % 128, lr[valid] // 128, (r[valid] // HW) - q0] = \
            1.0 / (HW * K * qn[lr[valid]])
        am16 = amask.reshape(128, MT * SLOTS).view(np.float16)
        blob = np.concatenate(
            [zqt_dev[:, 0:128], bank_dev[:, 0:WPAD], zqt_dev[:, 128:],
             bank_dev[:, WPAD:], am16], axis=1)
        blob_devs.append(np.ascontiguousarray(blob))
    return blob_devs


def kernel(support_images, support_labels, query_images):
    from concourse import bass_utils

    if 'nc' not in _CACHE:
        _CACHE['nc'] = _build_program()
    nc = _CACHE['nc']

    blob_devs = _host_prep(support_images, support_labels, query_images)

    in_maps = [{'blob': blob_devs[c]} for c in range(NCORES)]
    try:
        res = bass_utils.run_bass_kernel_spmd(
            nc, in_maps, core_ids=list(range(NCORES)))
    except Exception:
        # transient NRT/tunnel failures happen; one retry
        import time
        time.sleep(2.0)
        res = bass_utils.run_bass_kernel_spmd(
            nc, in_maps, core_ids=list(range(NCORES)))
    scores = np.zeros((Q, WAYS), np.float32)
    for c in range(NCORES):
        q0 = (c * RPC) // HW
        part = res.results[c]['scores'].reshape(SLOTS, WAYS, MT).sum(2)
        for s in range(SLOTS):
            if q0 + s < Q:
                scores[q0 + s] += part[s]
    return scores.astype(np.float32)
